# revision 10
# baseline (speedup 1.0000x reference)
"""Sigmoid-attention (DiffAttention) kernel for 8 Trainium2 NeuronCores.

Problem:  N=L=4096, H=8 heads, M=D=64.
    scores[n,l,h] = sigmoid(q[n,h,:] . k[l,h,:])
    out[n,h,:]    = (scores @ v) / sum_l(scores)        (per head)

Sharding: one head per core (8 heads == 8 cores). Each core gets its
head's Q/K transposed to [64, 4096] (duplicated onto both SBUF
partition halves) plus V packed as [V | ones] tiles, computes the full
attention for that head, and returns the head output transposed
([64, 4096]); the host restores [4096, 8, 64].

The Scalar (ACT) engine's sigmoid is the bottleneck (1 elem/cycle/lane
@1.2GHz ~= 109us/core floor), so:
  - n-chunks of 512; sigmoid groups of 3 l-tiles -> one ACTIVATE over
    [128, 1536] fp32 PSUM -> fp16 SBUF (amortizes per-instr overhead).
  - groups 2/6/10 of each chunk run their "sigmoid" on the (otherwise
    idle) Vector engine instead, as two custom fused DVE ops
    evaluating a clipped odd deg-9 polynomial (max err 2.7e-3):
      op A: s=x^2; W = (((a*s+b)*s+c)*s+1)*s          (= p_even(s)/c3)
      op B: g = clip((W*c3 + c1)*x, -0.5, 0.5)        (= sigma - 0.5)
    Those tiles hold g = sigma-0.5; the missing 0.5 is restored in the
    epilogue via a host-precomputed per-d correction 0.5*sum_{l in S} v
    (S, the DVE l-subset, is the same for every chunk).
  - mm1: S^T[l,n] = matmul(lhsT=K^T block, rhs=Q^T chunk), even
    l-tiles on PE row group 0, odd on row group 64 (concurrent pairs).
  - mm2: one 128-contraction matmul per l-tile,
    acc[65,512] += [V|1]^T @ A^T, single accumulator (1 PSUM bank).
  - PSUM: 2 x sT[128,1536] (3 banks) + 2 x acc[65,512] (1 bank) = 8.
  - mm2 lags the sigmoid by 3 groups (software pipeline across chunk
    boundaries) so neither sigmoid engine ever waits on the PE.
"""

from contextlib import ExitStack

import numpy as np

import concourse.bass as bass
import concourse.mybir as mybir
import concourse.tile as tile
from concourse import bacc
from concourse.bass import ts
from concourse.bass_utils import run_bass_kernel_spmd

import concourse.dve_ops as dve_ops_mod
from concourse.dve_ops import DveOp
from concourse.dve_spec import (
    C0,
    C1,
    C2,
    One,
    Spec,
    Src0,
    Src1,
    Zero,
    _has_src1,
    lower,
    maxx,
    minn,
    sq,
)
from concourse.dve_uop import DveOpSpec

N, L, H, M, D = 4096, 4096, 8, 64, 64
NCORES = 8
NCHUNK = 512  # n columns per chunk (acc free dim)
NCHUNKS = N // NCHUNK
LTILES = L // 128
GRP = 2  # l-tiles per sigmoid group
VW = D + 1  # V columns + ones column
SKEW = 5  # mm2 lags sigmoid by this many groups
CDT = mybir.dt.float16  # PE input dtype
FP32 = mybir.dt.float32
SIGMOID = mybir.ActivationFunctionType.Sigmoid

# deg-9 odd minimax fit of sigmoid(x)-0.5 on [0,6] (numpy weighted-lstsq
# Remez; max abs err 2.72e-3 over the whole real line after the clip).
PC1 = 0.24396364390850067
PC3 = -0.015843460336327553
PC5 = 0.0007671478670090437
PC7 = -1.9324557797517627e-05
PC9 = 1.8882809627029928e-07

_CACHE: dict = {}

GROUPS = []
_lt = 0
while _lt < LTILES:
    GROUPS.append(list(range(_lt, min(_lt + GRP, LTILES))))
    _lt += GRP
NGRP = len(GROUPS)
NT = NCHUNKS * NGRP

DVE_GIS = (1, 5, 9, 13)  # groups whose sigmoid runs on the Vector engine
DVE_LTILES = [lt for gi in DVE_GIS for lt in GROUPS[gi]]


# ---- custom fused DVE ops (registered into concourse.dve_ops) ----


def _ref_sig_poly_a(in0, in1, s0, s1, imm2):
    f = np.float32
    s = (in0 * in0).astype(f)
    u = ((f(s0) * s).astype(f) + f(s1)).astype(f)
    u = ((u * s).astype(f) + f(imm2)).astype(f)
    u = ((u * s).astype(f) + f(1.0)).astype(f)
    return (u * s).astype(f)


def _ref_sig_poly_b(in0, in1, s0, s1, imm2):
    f = np.float32
    a = ((in1 * f(s0)).astype(f) + f(s1)).astype(f)
    g = (a * in0).astype(f)
    return np.minimum(np.maximum(g, -f(imm2)), f(imm2))


def _make_op(name, spec):
    shas = {}
    for ver in ("v3", "v4"):
        uops = lower(spec, ver=ver)
        shas[ver] = DveOpSpec(
            name=name, opcode=1, uops=uops, rd1_en=_has_src1(spec)
        ).sha(ver)
    return DveOp(name, spec, False, shas)


def _register(op):
    if op.name in dve_ops_mod._SUB_OPCODE_FOR_NAME:
        return
    dve_ops_mod.OPS.append(op)
    dve_ops_mod._SUB_OPCODE_FOR_NAME[op.name] = (
        dve_ops_mod._CUSTOM_DVE_ROW_BASE + len(dve_ops_mod.OPS) - 1
    )
    dve_ops_mod.CUSTOM_DVE_SPECS[op.name] = op.spec


_s1 = sq(Src0)
SIG_POLY_A = _make_op(
    "SIG_POLY_A_ANT",
    Spec(
        body=((((C0 * _s1 + C1) * _s1 + C2) * _s1 + One) * _s1),
        reference=_ref_sig_poly_a,
    ),
)
SIG_POLY_B = _make_op(
    "SIG_POLY_B_ANT",
    Spec(
        body=minn(maxx((Src1 * C0 + C1) * Src0, Zero - C2), C2),
        reference=_ref_sig_poly_b,
    ),
)
_register(SIG_POLY_A)
_register(SIG_POLY_B)


def build_nc():
    nc = bacc.Bacc("TRN2", target_bir_lowering=False, debug=False)

    q2_d = nc.dram_tensor("q2", [128, N], CDT, kind="ExternalInput").ap()
    k2_d = nc.dram_tensor("k2", [128, L], CDT, kind="ExternalInput").ap()
    v1_d = nc.dram_tensor("v1", [128, LTILES * VW], CDT, kind="ExternalInput").ap()
    corr_d = nc.dram_tensor("corr", [VW, 1], FP32, kind="ExternalInput").ap()
    out_d = nc.dram_tensor("out", [D, N], FP32, kind="ExternalOutput").ap()

    with ExitStack() as ctx:
        tc = ctx.enter_context(tile.TileContext(nc))
        const = ctx.enter_context(tc.tile_pool(name="const", bufs=1))
        apool = ctx.enter_context(tc.tile_pool(name="apool", bufs=SKEW + 5))
        hpool = ctx.enter_context(tc.tile_pool(name="hpool", bufs=2))
        io = ctx.enter_context(tc.tile_pool(name="io", bufs=2))
        psS = ctx.enter_context(tc.tile_pool(name="psS", bufs=3, space="PSUM"))
        psAcc = ctx.enter_context(tc.tile_pool(name="psAcc", bufs=2, space="PSUM"))

        q2_s = const.tile([128, N], CDT)
        k2_s = const.tile([128, L], CDT)
        v1_s = const.tile([128, LTILES * VW], CDT)
        corr_s = const.tile([VW, 1], FP32)
        # Partition-split first loads: each dma_start lowers to one
        # per-partition descriptor chain on a single DMA engine, so halving
        # the partition range halves the serial latency of the critical
        # pieces (k/q for group 0, V for the first mm2s).
        halves = ((0, 64), (64, 128))
        quarters = tuple((i * 32, (i + 1) * 32) for i in range(4))
        for lo, hi in quarters:
            nc.sync.dma_start(out=k2_s[lo:hi, 0:512], in_=k2_d[lo:hi, 0:512])
        for lo, hi in quarters:
            nc.sync.dma_start(out=q2_s[lo:hi, 0:512], in_=q2_d[lo:hi, 0:512])
        for lo, hi in halves:
            nc.sync.dma_start(out=v1_s[lo:hi, :], in_=v1_d[lo:hi, :])
        for cs in range(512, L, 512):
            for lo, hi in halves:
                nc.sync.dma_start(
                    out=k2_s[lo:hi, cs : cs + 512], in_=k2_d[lo:hi, cs : cs + 512]
                )
        for cs in range(512, N, 512):
            nc.sync.dma_start(out=q2_s[:, cs : cs + 512], in_=q2_d[:, cs : cs + 512])
        nc.sync.dma_start(out=corr_s, in_=corr_d)

        def mm1sig(ci, gi):
            cs = ci * NCHUNK
            g = GROUPS[gi]
            w = len(g) * NCHUNK
            sT = psS.tile([128, GRP * NCHUNK], FP32, tag="sT", name="sT")
            for j, lt in enumerate(g):
                # contraction over both duplicated halves computes 2*(q.k);
                # the sigmoid scale (and rescaled poly constants) undo it.
                nc.tensor.matmul(
                    sT[:, ts(j, NCHUNK)],
                    k2_s[:, ts(lt, 128)],
                    q2_s[:, cs : cs + NCHUNK],
                    start=True,
                    stop=True,
                )
            aT = apool.tile([128, GRP * NCHUNK], CDT, tag="aT", name="aT")
            if gi in DVE_GIS:
                h2 = hpool.tile([128, GRP * NCHUNK], FP32, tag="h2", name="h2")
                nc.vector._custom_dve(
                    SIG_POLY_A,
                    out=h2[:, 0:w],
                    in0=sT[:, 0:w],
                    s0=PC9 / PC3 / 64.0,
                    s1=PC7 / PC3 / 16.0,
                    imm2=PC5 / PC3 / 4.0,
                )
                nc.vector._custom_dve(
                    SIG_POLY_B,
                    out=aT[:, 0:w],
                    in0=sT[:, 0:w],
                    in1=h2[:, 0:w],
                    s0=PC3 / 8.0,
                    s1=PC1 / 2.0,
                    imm2=0.5,
                )
            else:
                nc.scalar.activation(aT[:, 0:w], sT[:, 0:w], SIGMOID, scale=0.5)
            return aT

        def mm2(gi, aT, acc):
            for j, lt in enumerate(GROUPS[gi]):
                nc.tensor.matmul(
                    acc,
                    v1_s[:, lt * VW : (lt + 1) * VW],
                    aT[:, ts(j, NCHUNK)],
                    start=(lt == 0),
                    stop=(lt == LTILES - 1),
                )

        def epilogue1(ci, acc):
            # summ = acc + corr restores the 0.5 dropped by the DVE tiles
            # (numerator rows) and their count/2 (normalizer row).
            summ = io.tile([VW, NCHUNK], FP32, tag="summ")
            nc.vector.tensor_scalar_add(summ, acc, corr_s)
            norm_sb = io.tile([1, NCHUNK], FP32, tag="norm")
            nc.vector.tensor_copy(norm_sb, summ[D : D + 1, :])
            bc = io.tile([D, NCHUNK], FP32, tag="bc")
            nc.gpsimd.partition_broadcast(bc, norm_sb, channels=D)
            return summ, bc

        def epilogue2(ci, summ, bc):
            # Emitted one group after epilogue1 so the gpsimd broadcast never
            # head-blocks the Vector FIFO (a sig group runs in between).
            cs = ci * NCHUNK
            rec = io.tile([D, NCHUNK], FP32, tag="rec")
            nc.vector.reciprocal_approx_fast(out=rec, in_=bc)
            o = io.tile([D, NCHUNK], FP32, tag="o")
            nc.gpsimd.tensor_mul(o, summ[0:D, :], rec)
            for lo in range(0, D, 16):
                nc.sync.dma_start(
                    out=out_d[lo : lo + 16, cs : cs + NCHUNK], in_=o[lo : lo + 16, :]
                )

        hist = {}
        acc = None
        pend2 = None
        for t in range(NT):
            ci, gi = divmod(t, NGRP)
            if gi == 0:
                acc = psAcc.tile([VW, NCHUNK], FP32, tag="acc")
            hist[t] = (ci, gi, mm1sig(ci, gi), acc)
            if pend2 is not None:
                epilogue2(*pend2)
                pend2 = None
            if t - SKEW in hist:
                pci, pgi, paT, pacc = hist.pop(t - SKEW)
                mm2(pgi, paT, pacc)
                if pgi == NGRP - 1:
                    pend2 = (pci, *epilogue1(pci, pacc))
        for t in sorted(hist):
            pci, pgi, paT, pacc = hist[t]
            mm2(pgi, paT, pacc)
            if pend2 is not None:
                epilogue2(*pend2)
                pend2 = None
            if pgi == NGRP - 1:
                pend2 = (pci, *epilogue1(pci, pacc))
        if pend2 is not None:
            epilogue2(*pend2)

    nc.compile()
    return nc


def get_nc():
    if "nc" not in _CACHE:
        _CACHE["nc"] = build_nc()
    return _CACHE["nc"]


def make_in_maps(queries, keys, values):
    np_cdt = mybir.dt.np(CDT)
    dve_ls = np.array(
        [lt * 128 + p for lt in DVE_LTILES for p in range(128)], dtype=np.int64
    )
    in_maps = []
    for h in range(NCORES):
        qT = np.ascontiguousarray(queries[:, h, :].T.astype(np_cdt))
        kT = np.ascontiguousarray(keys[:, h, :].T.astype(np_cdt))
        v1 = np.empty((L, VW), np_cdt)
        v1[:, :D] = values[:, h, :]
        v1[:, D] = 1.0
        v1p = np.ascontiguousarray(
            v1.reshape(LTILES, 128, VW).transpose(1, 0, 2).reshape(128, LTILES * VW)
        )
        corr = np.empty((VW, 1), np.float32)
        corr[:D, 0] = 0.5 * values[dve_ls, h, :].astype(np.float64).sum(axis=0)
        corr[D, 0] = 0.5 * len(dve_ls)
        in_maps.append(
            {
                "q2": np.vstack([qT, qT]),
                "k2": np.vstack([kT, kT]),
                "v1": v1p,
                "corr": corr,
            }
        )
    return in_maps


def run(queries, keys, values, trace=False):
    """Returns (out [N,H,D] fp32, BassKernelResults)."""
    nc = get_nc()
    in_maps = make_in_maps(queries, keys, values)
    res = run_bass_kernel_spmd(nc, in_maps, core_ids=list(range(NCORES)), trace=trace)
    out = np.empty((N, H, D), np.float32)
    for h in range(NCORES):
        out[:, h, :] = res.results[h]["out"].T
    return out, res


def kernel(queries, keys, values):
    out, _ = run(np.asarray(queries), np.asarray(keys), np.asarray(values))
    return out


# revision 11
# speedup vs baseline: 1.6709x; 1.6709x over previous
"""Sigmoid-attention (DiffAttention) kernel for 8 Trainium2 NeuronCores.

Problem:  N=L=4096, H=8 heads, M=D=64.
    scores[n,l,h] = sigmoid(q[n,h,:] . k[l,h,:])
    out[n,h,:]    = (scores @ v) / sum_l(scores)        (per head)

Sharding: one head per core (8 heads == 8 cores). Each core gets its
head's Q/K transposed to [64, 4096] (duplicated onto both SBUF
partition halves) plus V packed as [V | ones] tiles, computes the full
attention for that head, and returns the head output transposed
([64, 4096]); the host restores [4096, 8, 64].

The Scalar (ACT) engine's sigmoid is the bottleneck (1 elem/cycle/lane
@1.2GHz ~= 109us/core floor), so:
  - n-chunks of 512; sigmoid groups of 3 l-tiles -> one ACTIVATE over
    [128, 1536] fp32 PSUM -> fp16 SBUF (amortizes per-instr overhead).
  - groups 2/6/10 of each chunk run their "sigmoid" on the (otherwise
    idle) Vector engine instead, as two custom fused DVE ops
    evaluating a clipped odd deg-9 polynomial (max err 2.7e-3):
      op A: s=x^2; W = (((a*s+b)*s+c)*s+1)*s          (= p_even(s)/c3)
      op B: g = clip((W*c3 + c1)*x, -0.5, 0.5)        (= sigma - 0.5)
    Those tiles hold g = sigma-0.5; the missing 0.5 is restored in the
    epilogue via a host-precomputed per-d correction 0.5*sum_{l in S} v
    (S, the DVE l-subset, is the same for every chunk).
  - mm1: S^T[l,n] = matmul(lhsT=K^T block, rhs=Q^T chunk), even
    l-tiles on PE row group 0, odd on row group 64 (concurrent pairs).
  - mm2: one 128-contraction matmul per l-tile,
    acc[65,512] += [V|1]^T @ A^T, single accumulator (1 PSUM bank).
  - PSUM: 2 x sT[128,1536] (3 banks) + 2 x acc[65,512] (1 bank) = 8.
  - mm2 lags the sigmoid by 3 groups (software pipeline across chunk
    boundaries) so neither sigmoid engine ever waits on the PE.
"""

from contextlib import ExitStack

import numpy as np

import concourse.bass as bass
import concourse.mybir as mybir
import concourse.tile as tile
from concourse import bacc
from concourse.bass import ts
from concourse.bass_utils import run_bass_kernel_spmd

import concourse.dve_ops as dve_ops_mod
from concourse.dve_ops import DveOp
from concourse.dve_spec import (
    C0,
    C1,
    C2,
    One,
    Spec,
    Src0,
    Src1,
    Zero,
    _has_src1,
    lower,
    maxx,
    minn,
    sq,
)
from concourse.dve_uop import DveOpSpec

N, L, H, M, D = 4096, 4096, 8, 64, 64
NCORES = 8
NCHUNK = 512  # n columns per chunk (acc free dim)
NCHUNKS = N // NCHUNK
LTILES = L // 128
GRP = 2  # l-tiles per sigmoid group
VW = D + 1  # V columns + ones column
SKEW = 5  # mm2 lags sigmoid by this many groups
CDT = mybir.dt.float16  # PE input dtype
FP32 = mybir.dt.float32
SIGMOID = mybir.ActivationFunctionType.Sigmoid

# deg-9 odd minimax fit of sigmoid(x)-0.5 on [0,6] (numpy weighted-lstsq
# Remez; max abs err 2.72e-3 over the whole real line after the clip).
PC1 = 0.24396364390850067
PC3 = -0.015843460336327553
PC5 = 0.0007671478670090437
PC7 = -1.9324557797517627e-05
PC9 = 1.8882809627029928e-07

_CACHE: dict = {}

GROUPS = []
_lt = 0
while _lt < LTILES:
    GROUPS.append(list(range(_lt, min(_lt + GRP, LTILES))))
    _lt += GRP
NGRP = len(GROUPS)
NT = NCHUNKS * NGRP

DVE_GIS = (1, 5, 9, 13)  # groups whose sigmoid runs on the Vector engine
DVE_LTILES = [lt for gi in DVE_GIS for lt in GROUPS[gi]]


# ---- custom fused DVE ops (registered into concourse.dve_ops) ----


def _ref_sig_poly_a(in0, in1, s0, s1, imm2):
    f = np.float32
    s = (in0 * in0).astype(f)
    u = ((f(s0) * s).astype(f) + f(s1)).astype(f)
    u = ((u * s).astype(f) + f(imm2)).astype(f)
    u = ((u * s).astype(f) + f(1.0)).astype(f)
    return (u * s).astype(f)


def _ref_sig_poly_b(in0, in1, s0, s1, imm2):
    f = np.float32
    a = ((in1 * f(s0)).astype(f) + f(s1)).astype(f)
    g = (a * in0).astype(f)
    return np.minimum(np.maximum(g, -f(imm2)), f(imm2))


def _make_op(name, spec):
    shas = {}
    for ver in ("v3", "v4"):
        uops = lower(spec, ver=ver)
        shas[ver] = DveOpSpec(
            name=name, opcode=1, uops=uops, rd1_en=_has_src1(spec)
        ).sha(ver)
    return DveOp(name, spec, False, shas)


def _register(op):
    if op.name in dve_ops_mod._SUB_OPCODE_FOR_NAME:
        return
    dve_ops_mod.OPS.append(op)
    dve_ops_mod._SUB_OPCODE_FOR_NAME[op.name] = (
        dve_ops_mod._CUSTOM_DVE_ROW_BASE + len(dve_ops_mod.OPS) - 1
    )
    dve_ops_mod.CUSTOM_DVE_SPECS[op.name] = op.spec


_s1 = sq(Src0)
SIG_POLY_A = _make_op(
    "SIG_POLY_A_ANT",
    Spec(
        body=((((C0 * _s1 + C1) * _s1 + C2) * _s1 + One) * _s1),
        reference=_ref_sig_poly_a,
    ),
)
SIG_POLY_B = _make_op(
    "SIG_POLY_B_ANT",
    Spec(
        body=minn(maxx((Src1 * C0 + C1) * Src0, Zero - C2), C2),
        reference=_ref_sig_poly_b,
    ),
)
_register(SIG_POLY_A)
_register(SIG_POLY_B)


def build_nc():
    nc = bacc.Bacc("TRN2", target_bir_lowering=False, debug=False)

    q2_d = nc.dram_tensor("q2", [128, N], CDT, kind="ExternalInput").ap()
    k2_d = nc.dram_tensor("k2", [128, L], CDT, kind="ExternalInput").ap()
    v1_d = nc.dram_tensor("v1", [128, LTILES * VW], CDT, kind="ExternalInput").ap()
    corr_d = nc.dram_tensor("corr", [VW, 1], FP32, kind="ExternalInput").ap()
    out_d = nc.dram_tensor("out", [D, N], FP32, kind="ExternalOutput").ap()

    with ExitStack() as ctx:
        tc = ctx.enter_context(tile.TileContext(nc))
        const = ctx.enter_context(tc.tile_pool(name="const", bufs=1))
        apool = ctx.enter_context(tc.tile_pool(name="apool", bufs=SKEW + 5))
        hpool = ctx.enter_context(tc.tile_pool(name="hpool", bufs=2))
        io = ctx.enter_context(tc.tile_pool(name="io", bufs=2))
        psS = ctx.enter_context(tc.tile_pool(name="psS", bufs=3, space="PSUM"))
        psAcc = ctx.enter_context(tc.tile_pool(name="psAcc", bufs=2, space="PSUM"))

        q2_s = const.tile([128, N], CDT)
        k2_s = const.tile([128, L], CDT)
        v1_s = const.tile([128, LTILES * VW], CDT)
        corr_s = const.tile([VW, 1], FP32)
        # Partition-split first loads: each dma_start lowers to one
        # per-partition descriptor chain on a single DMA engine, so halving
        # the partition range halves the serial latency of the critical
        # pieces (k/q for group 0, V for the first mm2s).
        halves = ((0, 64), (64, 128))
        quarters = tuple((i * 32, (i + 1) * 32) for i in range(4))
        for lo, hi in quarters:
            nc.sync.dma_start(out=k2_s[lo:hi, 0:512], in_=k2_d[lo:hi, 0:512])
        for lo, hi in quarters:
            nc.sync.dma_start(out=q2_s[lo:hi, 0:512], in_=q2_d[lo:hi, 0:512])
        for lo, hi in halves:
            nc.sync.dma_start(out=v1_s[lo:hi, :], in_=v1_d[lo:hi, :])
        for cs in range(512, L, 512):
            for lo, hi in halves:
                nc.sync.dma_start(
                    out=k2_s[lo:hi, cs : cs + 512], in_=k2_d[lo:hi, cs : cs + 512]
                )
        for cs in range(512, N, 512):
            nc.sync.dma_start(out=q2_s[:, cs : cs + 512], in_=q2_d[:, cs : cs + 512])
        nc.sync.dma_start(out=corr_s, in_=corr_d)

        def mm1sig(ci, gi):
            cs = ci * NCHUNK
            g = GROUPS[gi]
            w = len(g) * NCHUNK
            sT = psS.tile([128, GRP * NCHUNK], FP32, tag="sT", name="sT")
            for j, lt in enumerate(g):
                # contraction over both duplicated halves computes 2*(q.k);
                # the sigmoid scale (and rescaled poly constants) undo it.
                nc.tensor.matmul(
                    sT[:, ts(j, NCHUNK)],
                    k2_s[:, ts(lt, 128)],
                    q2_s[:, cs : cs + NCHUNK],
                    start=True,
                    stop=True,
                )
            aT = apool.tile([128, GRP * NCHUNK], CDT, tag="aT", name="aT")
            if gi in DVE_GIS:
                h2 = hpool.tile([128, GRP * NCHUNK], FP32, tag="h2", name="h2")
                nc.vector._custom_dve(
                    SIG_POLY_A,
                    out=h2[:, 0:w],
                    in0=sT[:, 0:w],
                    s0=PC9 / PC3 / 64.0,
                    s1=PC7 / PC3 / 16.0,
                    imm2=PC5 / PC3 / 4.0,
                )
                nc.vector._custom_dve(
                    SIG_POLY_B,
                    out=aT[:, 0:w],
                    in0=sT[:, 0:w],
                    in1=h2[:, 0:w],
                    s0=PC3 / 8.0,
                    s1=PC1 / 2.0,
                    imm2=0.5,
                )
            else:
                nc.scalar.activation(aT[:, 0:w], sT[:, 0:w], SIGMOID, scale=0.5)
            return aT

        def mm2(gi, aT, acc):
            for j, lt in enumerate(GROUPS[gi]):
                nc.tensor.matmul(
                    acc,
                    v1_s[:, lt * VW : (lt + 1) * VW],
                    aT[:, ts(j, NCHUNK)],
                    start=(lt == 0),
                    stop=(lt == LTILES - 1),
                )

        def epilogue1(ci, acc):
            # summ = acc + corr restores the 0.5 dropped by the DVE tiles
            # (numerator rows) and their count/2 (normalizer row).
            summ = io.tile([VW, NCHUNK], FP32, tag="summ")
            nc.vector.tensor_scalar_add(summ, acc, corr_s)
            norm_sb = io.tile([1, NCHUNK], FP32, tag="norm")
            nc.vector.tensor_copy(norm_sb, summ[D : D + 1, :])
            bc = io.tile([D, NCHUNK], FP32, tag="bc")
            nc.gpsimd.partition_broadcast(bc, norm_sb, channels=D)
            return summ, bc

        def epilogue2(ci, summ, bc):
            # Emitted one group after epilogue1 so the gpsimd broadcast never
            # head-blocks the Vector FIFO (a sig group runs in between).
            cs = ci * NCHUNK
            rec = io.tile([D, NCHUNK], FP32, tag="rec")
            nc.vector.reciprocal_approx_fast(out=rec, in_=bc)
            o = io.tile([D, NCHUNK], FP32, tag="o")
            nc.vector.tensor_mul(o, summ[0:D, :], rec)
            for lo in range(0, D, 16):
                nc.sync.dma_start(
                    out=out_d[lo : lo + 16, cs : cs + NCHUNK], in_=o[lo : lo + 16, :]
                )

        hist = {}
        acc = None
        pend2 = None
        for t in range(NT):
            ci, gi = divmod(t, NGRP)
            if gi == 0:
                acc = psAcc.tile([VW, NCHUNK], FP32, tag="acc")
            hist[t] = (ci, gi, mm1sig(ci, gi), acc)
            if pend2 is not None:
                epilogue2(*pend2)
                pend2 = None
            if t - SKEW in hist:
                pci, pgi, paT, pacc = hist.pop(t - SKEW)
                mm2(pgi, paT, pacc)
                if pgi == NGRP - 1:
                    pend2 = (pci, *epilogue1(pci, pacc))
        for t in sorted(hist):
            pci, pgi, paT, pacc = hist[t]
            mm2(pgi, paT, pacc)
            if pend2 is not None:
                epilogue2(*pend2)
                pend2 = None
            if pgi == NGRP - 1:
                pend2 = (pci, *epilogue1(pci, pacc))
        if pend2 is not None:
            epilogue2(*pend2)

    nc.compile()
    return nc


def get_nc():
    if "nc" not in _CACHE:
        _CACHE["nc"] = build_nc()
    return _CACHE["nc"]


def make_in_maps(queries, keys, values):
    np_cdt = mybir.dt.np(CDT)
    dve_ls = np.array(
        [lt * 128 + p for lt in DVE_LTILES for p in range(128)], dtype=np.int64
    )
    in_maps = []
    for h in range(NCORES):
        qT = np.ascontiguousarray(queries[:, h, :].T.astype(np_cdt))
        kT = np.ascontiguousarray(keys[:, h, :].T.astype(np_cdt))
        v1 = np.empty((L, VW), np_cdt)
        v1[:, :D] = values[:, h, :]
        v1[:, D] = 1.0
        v1p = np.ascontiguousarray(
            v1.reshape(LTILES, 128, VW).transpose(1, 0, 2).reshape(128, LTILES * VW)
        )
        corr = np.empty((VW, 1), np.float32)
        corr[:D, 0] = 0.5 * values[dve_ls, h, :].astype(np.float64).sum(axis=0)
        corr[D, 0] = 0.5 * len(dve_ls)
        in_maps.append(
            {
                "q2": np.vstack([qT, qT]),
                "k2": np.vstack([kT, kT]),
                "v1": v1p,
                "corr": corr,
            }
        )
    return in_maps


def run(queries, keys, values, trace=False):
    """Returns (out [N,H,D] fp32, BassKernelResults)."""
    nc = get_nc()
    in_maps = make_in_maps(queries, keys, values)
    res = run_bass_kernel_spmd(nc, in_maps, core_ids=list(range(NCORES)), trace=trace)
    out = np.empty((N, H, D), np.float32)
    for h in range(NCORES):
        out[:, h, :] = res.results[h]["out"].T
    return out, res


def kernel(queries, keys, values):
    out, _ = run(np.asarray(queries), np.asarray(keys), np.asarray(values))
    return out


# revision 12
# speedup vs baseline: 1.7181x; 1.0282x over previous
"""Sigmoid-attention (DiffAttention) kernel for 8 Trainium2 NeuronCores.

Problem:  N=L=4096, H=8 heads, M=D=64.
    scores[n,l,h] = sigmoid(q[n,h,:] . k[l,h,:])
    out[n,h,:]    = (scores @ v) / sum_l(scores)        (per head)

Sharding: one head per core (8 heads == 8 cores). Each core gets its
head's Q/K transposed to [64, 4096] (duplicated onto both SBUF
partition halves) plus V packed as [V | ones] tiles, computes the full
attention for that head, and returns the head output transposed
([64, 4096]); the host restores [4096, 8, 64].

The Scalar (ACT) engine's sigmoid is the bottleneck (1 elem/cycle/lane
@1.2GHz ~= 109us/core floor), so:
  - n-chunks of 512; sigmoid groups of 3 l-tiles -> one ACTIVATE over
    [128, 1536] fp32 PSUM -> fp16 SBUF (amortizes per-instr overhead).
  - groups 2/6/10 of each chunk run their "sigmoid" on the (otherwise
    idle) Vector engine instead, as two custom fused DVE ops
    evaluating a clipped odd deg-9 polynomial (max err 2.7e-3):
      op A: s=x^2; W = (((a*s+b)*s+c)*s+1)*s          (= p_even(s)/c3)
      op B: g = clip((W*c3 + c1)*x, -0.5, 0.5)        (= sigma - 0.5)
    Those tiles hold g = sigma-0.5; the missing 0.5 is restored in the
    epilogue via a host-precomputed per-d correction 0.5*sum_{l in S} v
    (S, the DVE l-subset, is the same for every chunk).
  - mm1: S^T[l,n] = matmul(lhsT=K^T block, rhs=Q^T chunk), even
    l-tiles on PE row group 0, odd on row group 64 (concurrent pairs).
  - mm2: one 128-contraction matmul per l-tile,
    acc[65,512] += [V|1]^T @ A^T, single accumulator (1 PSUM bank).
  - PSUM: 2 x sT[128,1536] (3 banks) + 2 x acc[65,512] (1 bank) = 8.
  - mm2 lags the sigmoid by 3 groups (software pipeline across chunk
    boundaries) so neither sigmoid engine ever waits on the PE.
"""

from contextlib import ExitStack

import numpy as np

import concourse.bass as bass
import concourse.mybir as mybir
import concourse.tile as tile
from concourse import bacc
from concourse.bass import ts
from concourse.bass_utils import run_bass_kernel_spmd

import concourse.dve_ops as dve_ops_mod
from concourse.dve_ops import DveOp
from concourse.dve_spec import (
    C0,
    C1,
    C2,
    One,
    Spec,
    Src0,
    Src1,
    Zero,
    _has_src1,
    lower,
    maxx,
    minn,
    sq,
)
from concourse.dve_uop import DveOpSpec

N, L, H, M, D = 4096, 4096, 8, 64, 64
NCORES = 8
NCHUNK = 512  # n columns per chunk (acc free dim)
NCHUNKS = N // NCHUNK
LTILES = L // 128
GRP = 2  # l-tiles per sigmoid group
VW = D + 1  # V columns + ones column
SKEW = 5  # mm2 lags sigmoid by this many groups
CDT = mybir.dt.float16  # PE input dtype
FP32 = mybir.dt.float32
SIGMOID = mybir.ActivationFunctionType.Sigmoid

# deg-9 odd minimax fit of sigmoid(x)-0.5 on [0,6] (numpy weighted-lstsq
# Remez; max abs err 2.72e-3 over the whole real line after the clip).
PC1 = 0.24396364390850067
PC3 = -0.015843460336327553
PC5 = 0.0007671478670090437
PC7 = -1.9324557797517627e-05
PC9 = 1.8882809627029928e-07

_CACHE: dict = {}

GROUPS = []
_lt = 0
while _lt < LTILES:
    GROUPS.append(list(range(_lt, min(_lt + GRP, LTILES))))
    _lt += GRP
NGRP = len(GROUPS)
NT = NCHUNKS * NGRP

DVE_GIS = (1, 5, 9, 13)  # groups whose sigmoid runs on the Vector engine
DVE_LTILES = [lt for gi in DVE_GIS for lt in GROUPS[gi]]


# ---- custom fused DVE ops (registered into concourse.dve_ops) ----


def _ref_sig_poly_a(in0, in1, s0, s1, imm2):
    f = np.float32
    s = (in0 * in0).astype(f)
    u = ((f(s0) * s).astype(f) + f(s1)).astype(f)
    u = ((u * s).astype(f) + f(imm2)).astype(f)
    u = ((u * s).astype(f) + f(1.0)).astype(f)
    return (u * s).astype(f)


def _ref_sig_poly_b(in0, in1, s0, s1, imm2):
    f = np.float32
    a = ((in1 * f(s0)).astype(f) + f(s1)).astype(f)
    g = (a * in0).astype(f)
    return np.minimum(np.maximum(g, -f(imm2)), f(imm2))


def _make_op(name, spec):
    shas = {}
    for ver in ("v3", "v4"):
        uops = lower(spec, ver=ver)
        shas[ver] = DveOpSpec(
            name=name, opcode=1, uops=uops, rd1_en=_has_src1(spec)
        ).sha(ver)
    return DveOp(name, spec, False, shas)


def _register(op):
    if op.name in dve_ops_mod._SUB_OPCODE_FOR_NAME:
        return
    dve_ops_mod.OPS.append(op)
    dve_ops_mod._SUB_OPCODE_FOR_NAME[op.name] = (
        dve_ops_mod._CUSTOM_DVE_ROW_BASE + len(dve_ops_mod.OPS) - 1
    )
    dve_ops_mod.CUSTOM_DVE_SPECS[op.name] = op.spec


_s1 = sq(Src0)
SIG_POLY_A = _make_op(
    "SIG_POLY_A_ANT",
    Spec(
        body=((((C0 * _s1 + C1) * _s1 + C2) * _s1 + One) * _s1),
        reference=_ref_sig_poly_a,
    ),
)
SIG_POLY_B = _make_op(
    "SIG_POLY_B_ANT",
    Spec(
        body=minn(maxx((Src1 * C0 + C1) * Src0, Zero - C2), C2),
        reference=_ref_sig_poly_b,
    ),
)
_register(SIG_POLY_A)
_register(SIG_POLY_B)


def build_nc():
    nc = bacc.Bacc("TRN2", target_bir_lowering=False, debug=False)

    q2_d = nc.dram_tensor("q2", [128, N], CDT, kind="ExternalInput").ap()
    k2_d = nc.dram_tensor("k2", [128, L], CDT, kind="ExternalInput").ap()
    v1_d = nc.dram_tensor("v1", [128, LTILES * VW], CDT, kind="ExternalInput").ap()
    corr_d = nc.dram_tensor("corr", [VW, 1], FP32, kind="ExternalInput").ap()
    out_d = nc.dram_tensor("out", [D, N], FP32, kind="ExternalOutput").ap()

    with ExitStack() as ctx:
        tc = ctx.enter_context(tile.TileContext(nc))
        const = ctx.enter_context(tc.tile_pool(name="const", bufs=1))
        apool = ctx.enter_context(tc.tile_pool(name="apool", bufs=SKEW + 5))
        hpool = ctx.enter_context(tc.tile_pool(name="hpool", bufs=2))
        io = ctx.enter_context(tc.tile_pool(name="io", bufs=2))
        psS = ctx.enter_context(tc.tile_pool(name="psS", bufs=3, space="PSUM"))
        psAcc = ctx.enter_context(tc.tile_pool(name="psAcc", bufs=2, space="PSUM"))

        q2_s = const.tile([128, N], CDT)
        k2_s = const.tile([128, L], CDT)
        v1_s = const.tile([128, LTILES * VW], CDT)
        corr_s = const.tile([VW, 1], FP32)
        # Partition-split first loads: each dma_start lowers to one
        # per-partition descriptor chain on a single DMA engine, so halving
        # the partition range halves the serial latency of the critical
        # pieces (k/q for group 0, V for the first mm2s).
        halves = ((0, 64), (64, 128))
        quarters = tuple((i * 32, (i + 1) * 32) for i in range(4))
        for lo, hi in quarters:
            nc.sync.dma_start(out=k2_s[lo:hi, 0:512], in_=k2_d[lo:hi, 0:512])
        for lo, hi in quarters:
            nc.sync.dma_start(out=q2_s[lo:hi, 0:512], in_=q2_d[lo:hi, 0:512])
        for lo, hi in halves:
            nc.sync.dma_start(out=v1_s[lo:hi, :], in_=v1_d[lo:hi, :])
        for cs in range(512, L, 512):
            for lo, hi in halves:
                nc.sync.dma_start(
                    out=k2_s[lo:hi, cs : cs + 512], in_=k2_d[lo:hi, cs : cs + 512]
                )
        for cs in range(512, N, 512):
            nc.sync.dma_start(out=q2_s[:, cs : cs + 512], in_=q2_d[:, cs : cs + 512])
        nc.sync.dma_start(out=corr_s, in_=corr_d)

        def mm1sig(ci, gi):
            cs = ci * NCHUNK
            g = GROUPS[gi]
            w = len(g) * NCHUNK
            sT = psS.tile([128, GRP * NCHUNK], FP32, tag="sT", name="sT")
            for j, lt in enumerate(g):
                # even l-tiles on PE rows 0-63, odd on 64-127: the two
                # matmuls of a group stream concurrently (row tiling).
                half = slice(0, 64) if lt % 2 == 0 else slice(64, 128)
                nc.tensor.matmul(
                    sT[:, ts(j, NCHUNK)],
                    k2_s[half, ts(lt, 128)],
                    q2_s[half, cs : cs + NCHUNK],
                    start=True,
                    stop=True,
                )
            aT = apool.tile([128, GRP * NCHUNK], CDT, tag="aT", name="aT")
            if gi in DVE_GIS:
                h2 = hpool.tile([128, GRP * NCHUNK], FP32, tag="h2", name="h2")
                nc.vector._custom_dve(
                    SIG_POLY_A,
                    out=h2[:, 0:w],
                    in0=sT[:, 0:w],
                    s0=PC9 / PC3,
                    s1=PC7 / PC3,
                    imm2=PC5 / PC3,
                )
                nc.vector._custom_dve(
                    SIG_POLY_B,
                    out=aT[:, 0:w],
                    in0=sT[:, 0:w],
                    in1=h2[:, 0:w],
                    s0=PC3,
                    s1=PC1,
                    imm2=0.5,
                )
            else:
                nc.scalar.activation(aT[:, 0:w], sT[:, 0:w], SIGMOID)
            return aT

        def mm2(gi, aT, acc):
            for j, lt in enumerate(GROUPS[gi]):
                nc.tensor.matmul(
                    acc,
                    v1_s[:, lt * VW : (lt + 1) * VW],
                    aT[:, ts(j, NCHUNK)],
                    start=(lt == 0),
                    stop=(lt == LTILES - 1),
                )

        def epilogue1(ci, acc):
            # summ = acc + corr restores the 0.5 dropped by the DVE tiles
            # (numerator rows) and their count/2 (normalizer row).
            summ = io.tile([VW, NCHUNK], FP32, tag="summ")
            nc.vector.tensor_scalar_add(summ, acc, corr_s)
            norm_sb = io.tile([1, NCHUNK], FP32, tag="norm")
            nc.vector.tensor_copy(norm_sb, summ[D : D + 1, :])
            bc = io.tile([D, NCHUNK], FP32, tag="bc")
            nc.gpsimd.partition_broadcast(bc, norm_sb, channels=D)
            return summ, bc

        def epilogue2(ci, summ, bc):
            # Emitted one group after epilogue1 so the gpsimd broadcast never
            # head-blocks the Vector FIFO (a sig group runs in between).
            cs = ci * NCHUNK
            rec = io.tile([D, NCHUNK], FP32, tag="rec")
            nc.vector.reciprocal_approx_fast(out=rec, in_=bc)
            o = io.tile([D, NCHUNK], FP32, tag="o")
            nc.vector.tensor_mul(o, summ[0:D, :], rec)
            for lo in range(0, D, 16):
                nc.sync.dma_start(
                    out=out_d[lo : lo + 16, cs : cs + NCHUNK], in_=o[lo : lo + 16, :]
                )

        hist = {}
        acc = None
        pend2 = None
        for t in range(NT):
            ci, gi = divmod(t, NGRP)
            if gi == 0:
                acc = psAcc.tile([VW, NCHUNK], FP32, tag="acc")
            hist[t] = (ci, gi, mm1sig(ci, gi), acc)
            if pend2 is not None:
                epilogue2(*pend2)
                pend2 = None
            if t - SKEW in hist:
                pci, pgi, paT, pacc = hist.pop(t - SKEW)
                mm2(pgi, paT, pacc)
                if pgi == NGRP - 1:
                    pend2 = (pci, *epilogue1(pci, pacc))
        for t in sorted(hist):
            pci, pgi, paT, pacc = hist[t]
            mm2(pgi, paT, pacc)
            if pend2 is not None:
                epilogue2(*pend2)
                pend2 = None
            if pgi == NGRP - 1:
                pend2 = (pci, *epilogue1(pci, pacc))
        if pend2 is not None:
            epilogue2(*pend2)

    nc.compile()
    return nc


def get_nc():
    if "nc" not in _CACHE:
        _CACHE["nc"] = build_nc()
    return _CACHE["nc"]


def make_in_maps(queries, keys, values):
    np_cdt = mybir.dt.np(CDT)
    dve_ls = np.array(
        [lt * 128 + p for lt in DVE_LTILES for p in range(128)], dtype=np.int64
    )
    in_maps = []
    for h in range(NCORES):
        qT = np.ascontiguousarray(queries[:, h, :].T.astype(np_cdt))
        kT = np.ascontiguousarray(keys[:, h, :].T.astype(np_cdt))
        v1 = np.empty((L, VW), np_cdt)
        v1[:, :D] = values[:, h, :]
        v1[:, D] = 1.0
        v1p = np.ascontiguousarray(
            v1.reshape(LTILES, 128, VW).transpose(1, 0, 2).reshape(128, LTILES * VW)
        )
        corr = np.empty((VW, 1), np.float32)
        corr[:D, 0] = 0.5 * values[dve_ls, h, :].astype(np.float64).sum(axis=0)
        corr[D, 0] = 0.5 * len(dve_ls)
        in_maps.append(
            {
                "q2": np.vstack([qT, qT]),
                "k2": np.vstack([kT, kT]),
                "v1": v1p,
                "corr": corr,
            }
        )
    return in_maps


def run(queries, keys, values, trace=False):
    """Returns (out [N,H,D] fp32, BassKernelResults)."""
    nc = get_nc()
    in_maps = make_in_maps(queries, keys, values)
    res = run_bass_kernel_spmd(nc, in_maps, core_ids=list(range(NCORES)), trace=trace)
    out = np.empty((N, H, D), np.float32)
    for h in range(NCORES):
        out[:, h, :] = res.results[h]["out"].T
    return out, res


def kernel(queries, keys, values):
    out, _ = run(np.asarray(queries), np.asarray(keys), np.asarray(values))
    return out
